# revision 42
# baseline (speedup 1.0000x reference)
"""Trainium2 Bass kernel for the quirky multi-head attention problem.

Math (per batch b, head a), faithful to the reference:
    K = x[b] @ W_K[a].T          # [S, H]
    Q = x[b] @ W_Q[a].T
    V = x[b] @ W_V[a].T
    s[c, C] = (K @ Q.T)[c, C] / sqrt(H)        rows c = "key" index
    valid iff C <= c (tril); softmax over C per row c
    E = exp(s) * tril            # no max-subtraction: |s| <= ~7, fp32-safe
    denom[c] = sum_C E[c, C]
    z[C, h] = sum_c E[c, C] * (V/denom)[c, h]  # = E.T @ (V/denom)
    out[b] += z @ W_O[a].T
Sharding: 8 cores = 2 batches x 4 head-pairs; host sums the four
head-pair partial outputs per batch.

Key device-side structure (per core, heads processed sequentially):
  - Score matmuls are K=64 (head dim) -> issued as PE row-tiles: kt/qt
    for the current head are DUPLICATED across partition halves (SBUF->
    SBUF DMA), and consecutive 512-col chunks alternate tile_position
    (0,0)/(64,0).  Disjoint row-tiles stream concurrently (~2x measured).
  - exp via ScalarE ACT on 1024-wide PSUM wave tiles (2 banks x 2 bufs)
    with fused per-row accumulation (softmax denominator).
  - z^T accumulates in PSUM across row blocks; chunk j lives at
    partition half (j < NCH/2 ? 0 : 64) so z^T fits in 4 banks.
  - zsb is partition-STACKED across heads (head h at partitions h*64..):
    chunks are bounced out of PSUM per-chunk as each finalizes and
    DMA-remapped; the output projection then contracts both heads in a
    single K=128 matmul per (chunk, e-block).
  - Head 0 sweeps row blocks groups-forward / blocks-reversed (so the
    first blocks after each projection group are the widest -> ACT gets
    fed), head 1 fully reversed (chunk j's z finalizes at block 4j ->
    zsb copy + output projection trickle through the whole sweep).
  - Matmul dtypes: fp16 operands for attention (4e-4 rel err), f32r for
    projections.  Z_LAG delays z matmuls so PE streams without stalling
    on the softmax chain; FILL_LDW issues dependency-free ldweights to
    keep the PE activity monitor from dropping the clock to half rate.
"""

import math

import numpy as np

B, S_FULL, E, A, H = 2, 4096, 512, 8, 64
N_CORES = 8
NEG_BIG = -1.0e9

import os as _os

ATTN_DT = _os.environ.get("ATTN_DT", "fp16")
PROJ_DT = _os.environ.get("PROJ_DT", "f32r")
FILL_LDW = int(_os.environ.get("FILL_LDW", "3"))
Z_LAG = int(_os.environ.get("Z_LAG", "2"))
Z_LAG_WIDE = int(_os.environ.get("Z_LAG_WIDE", "2"))
PANEL_BUFS = int(_os.environ.get("PANEL_BUFS", "4"))

_prog_cache = {}


def _build_program(S, attn_dt=None, proj_dt=None):
    import concourse.mybir as mybir
    import concourse.tile as tile
    from concourse import bacc

    attn_dt = attn_dt or ATTN_DT
    proj_dt = proj_dt or PROJ_DT
    f32 = mybir.dt.float32
    f32r = mybir.dt.float32r
    bf16 = mybir.dt.bfloat16
    fp16 = mybir.dt.float16
    att_store = {"bf16": bf16, "fp16": fp16, "f32r": f32r, "f32": f32}[attn_dt]
    z_store = {"bf16": bf16, "fp16": fp16, "f32r": fp16, "f32": f32}[attn_dt]
    proj_store = {"f32r": f32r, "f32": f32}[proj_dt]

    EC = E // 128            # e chunks (contraction for projections)
    NCB = S // 128           # row blocks
    NCH = S // 512           # C chunks per full row
    HALF = NCH // 2          # chunks per partition half of z^T
    assert NCH % 2 == 0

    nc = bacc.Bacc("TRN2", target_bir_lowering=False, debug=False)

    # x and the K/Q/V weights ship as fp16 (halves the HBM-bound input
    # load) and pre-tiled to [partition, ec, ...] so each loads in one DMA
    # per quarter / one per weight (the in-order DMA issue queue was
    # delaying the kt/qt duplication DMAs that gate the first scores)
    xT = nc.dram_tensor("xT", [128, EC, S], fp16, kind="ExternalInput")
    wk = nc.dram_tensor("wk", [128, EC, 128], fp16, kind="ExternalInput")
    wq = nc.dram_tensor("wq", [128, EC, 128], fp16, kind="ExternalInput")
    wv = nc.dram_tensor("wv", [128, EC, 128], fp16, kind="ExternalInput")
    wob = nc.dram_tensor("wob", [128, E], fp16, kind="ExternalInput")
    maskw = nc.dram_tensor("maskw", [128, 1152], fp16, kind="ExternalInput")
    outT = nc.dram_tensor("outT", [E, S], f32, kind="ExternalOutput")

    ExpF = mybir.ActivationFunctionType.Exp
    AxX = mybir.AxisListType.X
    AluAdd = mybir.AluOpType.add
    AluMult = mybir.AluOpType.mult
    AluBypass = mybir.AluOpType.bypass

    with tile.TileContext(nc) as tc:
        with (
            tc.tile_pool(name="singles", bufs=1) as singles,
            tc.tile_pool(name="panelp", bufs=PANEL_BUFS) as panelp,
            tc.tile_pool(name="dupp", bufs=1) as dupp,
            tc.tile_pool(name="small", bufs=8) as small,
            tc.tile_pool(name="bounce", bufs=2) as bouncep,
            tc.tile_pool(name="outst", bufs=4) as outst,
            tc.tile_pool(name="ps", bufs=2, space="PSUM") as ps,
            tc.tile_pool(name="zps", bufs=1, space="PSUM") as zps,
        ):
            # ---- load inputs ----
            xt = singles.tile([128, EC, S], fp16)
            wks = singles.tile([128, EC, 128], fp16)
            wqs = singles.tile([128, EC, 128], fp16)
            wvs = singles.tile([128, EC, 128], fp16)
            nc.sync.dma_start(out=wks, in_=wk[:, :, :])
            nc.sync.dma_start(out=wqs, in_=wq[:, :, :])
            nc.sync.dma_start(out=wvs, in_=wv[:, :, :])
            # 8 column-span DMAs: finer-grained arrival (projection of span
            # cc can start as soon as its span lands) and parallel DMA queues
            for q in range(8):
                qsl = slice(q * 512, (q + 1) * 512)
                nc.sync.dma_start(out=xt[:, :, qsl], in_=xT[:, :, qsl])
            wosb = singles.tile([128, E], fp16)
            nc.sync.dma_start(out=wosb, in_=wob[:, :])
            mskw = singles.tile([128, 1152], fp16)
            nc.sync.dma_start(out=mskw, in_=maskw[:, :])
            # bf16 always: the K=1 zeroing matmuls are invalid ISA in f32r.
            zero_t = singles.tile([1, 576], bf16)
            nc.vector.memset(zero_t, 0.0)

            # ---- projections ----
            # kt/qt hold BOTH heads (h0 partitions 0-63, h1 64-127); the
            # per-head duplicated ktd/qtd (both halves = current head) are
            # made by SBUF->SBUF DMA so score chunk pairs can issue on
            # row-tiles (0,0)/(64,0) concurrently.
            kt = singles.tile([128, S], att_store)
            qt = singles.tile([128, S], att_store)
            vsb = singles.tile([128, NCB, 128], f32)
            # stacked z (head h at partitions h*64..): fp16 to save SBUF; the
            # output projection is then an fp16 x fp16, K=128 matmul
            zstk = singles.tile([128, NCH * 512], att_store)

            # One shared duplication buffer: head0 fills partitions 64-127
            # (its T8 copy), head1 fills 0-63 (its T0 copy); each head's
            # other half is read straight out of kt/qt.
            ktd = dupp.tile([128, S], att_store, name="ktd")
            qtd = dupp.tile([128, S], att_store, name="qtd")

            def kq_unit(cc):
                # project kt/qt chunk cc + duplicate head0's half for tiling
                def run():
                    csl = slice(cc * 512, (cc + 1) * 512)
                    for dst, w in ((kt, wks), (qt, wqs)):
                        wt = ps.tile([128, 1024], f32, tag="wave", name="wt")
                        for ec in range(EC):
                            nc.tensor.matmul(
                                wt[:, :512], w[:, ec, :], xt[:, ec, csl],
                                start=(ec == 0), stop=(ec == EC - 1),
                            )
                        nc.vector.tensor_copy(dst[:, csl], wt[:, :512])
                    emit_dup(0, cc)
                return run

            def vsb_unit(cb):
                def run():
                    bsl = slice(cb * 128, (cb + 1) * 128)
                    wt = ps.tile([128, 1024], f32, tag="wave", name="wt")
                    for ec in range(EC):
                        nc.tensor.matmul(
                            wt[:, :128], xt[:, ec, bsl], wvs[:, ec, :],
                            start=(ec == 0), stop=(ec == EC - 1),
                        )
                    nc.vector.tensor_copy(vsb[:, cb, :], wt[:, :128])
                return run

            def emit_dup(h, cc):
                hsl = slice(h * 64, (h + 1) * 64)
                dsl = slice(64, 128) if h == 0 else slice(0, 64)
                csl = slice(cc * 512, (cc + 1) * 512)
                for s_, dst in ((kt, ktd), (qt, qtd)):
                    nc.sync.dma_start(out=dst[dsl, csl], in_=s_[hsl, csl])

            def dup_unit(h, cc):
                return lambda: emit_dup(h, cc)

            # ---- attention per head ----
            for h in range(2):
                hs = slice(h * 64, (h + 1) * 64)
                if h == 0:
                    # narrow groups first (chunks arrive with x quarters),
                    # widest 16 blocks last as one descending stream; within
                    # groups descending so the widest block of each group
                    # lands right after its projections.
                    order = (list(range(7, -1, -1)) + list(range(15, 7, -1))
                             + list(range(31, 15, -1)))
                else:
                    order = list(range(NCB - 1, -1, -1))

                zT = zps.tile([128, HALF * 512], f32, name="zT")
                # Low chunks live on the partition half matching this head's
                # zstk rows (h*64..) so the LAST chunks to finalize (the lows,
                # in both sweep orders) copy out partition-preserving.
                lo_off = h * 64
                hi_off = 64 - lo_off

                def poff_of(j):
                    return lo_off if j < HALF else hi_off

                # Zero the high-chunk half of each z bank: matmul start=True
                # clears has_written only on the partitions it writes, so the
                # high half must be explicitly reset each head (stale bits
                # from the other head's low chunks would make start=False
                # accumulate onto garbage).
                for k in range(HALF):
                    nc.tensor.matmul(
                        zT[hi_off:hi_off + 64, k * 512:(k + 1) * 512],
                        zero_t[:, :64], zero_t[:, 64:576],
                        start=True, stop=False, skip_group_check=True,
                    )

                first_cb = {}
                last_cb = {}
                for j in range(NCH):
                    part = [cb for cb in order if cb >= 4 * j]
                    first_cb[j] = part[0]
                    last_cb[j] = part[-1]

                def zsb_unit(zT_c, hh, j):
                    # zT chunk j (finalized) -> stacked zsb at partition hh*64
                    def run():
                        poff = poff_of(j)
                        col = (j % HALF) * 512
                        dst_p = hh * 64
                        jsl = slice(j * 512, (j + 1) * 512)
                        if poff == dst_p:
                            nc.vector.tensor_copy(
                                zstk[dst_p:dst_p + 64, jsl],
                                zT_c[poff:poff + 64, col:col + 512],
                            )
                        else:
                            bt = bouncep.tile([128, 512], att_store, name="bt")
                            nc.vector.tensor_copy(
                                bt[poff:poff + 64, :],
                                zT_c[poff:poff + 64, col:col + 512],
                            )
                            nc.sync.dma_start(
                                out=zstk[dst_p:dst_p + 64, jsl],
                                in_=bt[poff:poff + 64, :],
                            )
                    return run

                def outp_unit(j, ecn, zT_c, bank):
                    # alternate targets (dead zT bank / wave pool) and copy
                    # engines (DVE / ACT) so consecutive projections pipeline
                    # instead of serializing on one bank + one copy engine
                    def run():
                        jsl = slice(j * 512, (j + 1) * 512)
                        esl = slice(ecn * 128, (ecn + 1) * 128)
                        if bank is None:
                            wt = ps.tile([128, 1024], f32, tag="wave", name="wt")
                            wv = wt[:, :512]
                        else:
                            wv = zT_c[:, bank * 512:bank * 512 + 512]
                        nc.tensor.matmul(
                            wv, wosb[:, esl], zstk[:, jsl],
                            start=True, stop=True,
                        )
                        st = outst.tile([128, 512], f32, name="st")
                        if ecn % 2:
                            nc.scalar.copy(st, wv)
                        else:
                            nc.vector.tensor_copy(st, wv)
                        nc.sync.dma_start(out=outT[esl, jsl], in_=st)
                    return run

                def emit_z(item):
                    vt_i, panel_i, nch_i, cb_i = item
                    # interleave low/high chunks so consecutive z matmuls
                    # land on col-tiles (0,0)/(0,64) and stream concurrently
                    lows = list(range(min(nch_i, HALF)))
                    highs = list(range(HALF, nch_i))
                    zorder = []
                    for i_ in range(len(lows)):
                        zorder.append(lows[i_])
                        if i_ < len(highs):
                            zorder.append(highs[i_])
                    for j in zorder:
                        poff = poff_of(j)
                        col = (j % HALF) * 512
                        start = (j < HALF) and cb_i == first_cb[j]
                        stop = cb_i == last_cb[j]
                        nc.tensor.matmul(
                            zT[poff:poff + 64, col:col + 512],
                            vt_i,
                            panel_i[:, j * 512:(j + 1) * 512],
                            start=start, stop=stop,
                            skip_group_check=True,
                        )
                        if stop:
                            units.append(zsb_unit(zT, h, j))
                            if h == 1 and j < HALF:
                                # bank j is dead once the low chunk copies
                                # out (its high partner finished earlier in
                                # the reverse sweep): project both chunks
                                dead_banks.append(j % HALF)
                                k_ = 0
                                for jj in (j, j + HALF):
                                    for ecn in range(EC):
                                        # every third target: the wave pool
                                        # (scores are sparse this late)
                                        if k_ % 3 == 2:
                                            bank = None
                                        else:
                                            bank = dead_banks[k_ % len(dead_banks)]
                                        units.append(outp_unit(jj, ecn, zT, bank))
                                        k_ += 1

                pending = []
                units = []
                dead_banks = []
                par = [0]
                for oi, cb in enumerate(order):
                    if h == 0:
                        if oi == 0:
                            # quarter 0 up-front: first blocks need it now
                            for cc in (0, 1):
                                kq_unit(cc)()
                            for vb in range(8):
                                vsb_unit(vb)()
                            units.extend(kq_unit(cc) for cc in (2, 3))
                            units.extend(vsb_unit(vb) for vb in range(8, 16))
                        elif oi == 2:
                            # flood the remaining projections into the
                            # PE-bound prologue (narrow blocks starve ACT
                            # regardless); x quarters 2-3 have arrived
                            units.extend(kq_unit(cc) for cc in range(4, 8))
                            units.extend(vsb_unit(vb) for vb in range(16, 32))
                        elif oi == 16:
                            # prefetch head1's duplicated kt/qt in background
                            units.extend(dup_unit(1, cc) for cc in range(NCH))
                    c0 = cb * 128
                    nch = (c0 + 128 + 511) // 512
                    nwaves = (nch + 1) // 2
                    lastw = c0 + 128 - (nch - 1) * 512   # width of diag chunk
                    panel = panelp.tile([128, S], z_store, name="panel")
                    if lastw < 512:
                        # zero the diag chunk tail so z matmuls read zeros
                        nc.gpsimd.memset(
                            panel[:, (nch - 1) * 512 + lastw:nch * 512], 0.0
                        )
                    rsp = small.tile([128, 4], f32, name="rsp")
                    for wv_i in range(nwaves):
                        jlo = 2 * wv_i
                        jhi = min(jlo + 2, nch)
                        wt = ps.tile([128, 1024], f32, tag="wave", name="wt")
                        for j in range(jlo, jhi):
                            w_n = lastw if j == nch - 1 else 512
                            p = 64 * (par[0] % 2)
                            par[0] += 1
                            direct = p == h * 64
                            ksrc = kt if direct else ktd
                            qsrc = qt if direct else qtd
                            nc.tensor.matmul(
                                wt[:, (j - jlo) * 512:(j - jlo) * 512 + w_n],
                                ksrc[p:p + 64, c0:c0 + 128],
                                qsrc[p:p + 64, j * 512:j * 512 + w_n],
                                start=True, stop=True,
                            )
                        # diag wave: no ACT accumulation; GpSimd handles it
                        wlen = (jhi - jlo - 1) * 512 + (lastw if jhi == nch else 512)
                        nc.scalar.activation(
                            out=panel[:, jlo * 512:jlo * 512 + wlen],
                            in_=wt[:, :wlen],
                            func=ExpF,
                            scale=1.0 / math.sqrt(H),
                            accum_out=None if jhi == nch
                            else rsp[:, wv_i:wv_i + 1],
                        )
                    # 0/1 triangle mask post-exp on the fp16 panel (col c0 is
                    # the diagonal start), then DVE reduces the diag wave for
                    # its denominator partial (replaces the ACT accum there).
                    nc.vector.tensor_mul(
                        panel[:, c0:c0 + 128], panel[:, c0:c0 + 128],
                        mskw[:, 1024:1152],
                    )
                    dlo = (nwaves - 1) * 1024
                    nc.vector.tensor_reduce(
                        rsp[:, nwaves - 1:nwaves],
                        panel[:, dlo:c0 + 128], axis=AxX, op=AluAdd,
                    )
                    rden = small.tile([128, 1], f32, name="rden")
                    if nwaves > 1:
                        den = small.tile([128, 1], f32, name="den")
                        nc.vector.tensor_reduce(den, rsp[:, :nwaves], axis=AxX, op=AluAdd)
                        nc.vector.reciprocal(rden, den)
                    else:
                        nc.vector.reciprocal(rden, rsp[:, 0:1])
                    vt = small.tile([128, 64], z_store, name="vt")
                    nc.vector.tensor_scalar_mul(vt, vsb[:, cb, hs], rden)
                    pending.append((vt, panel, nch, cb))
                    lag = Z_LAG if nch <= 4 else Z_LAG_WIDE
                    while len(pending) > lag:
                        emit_z(pending.pop(0))
                    npop = 4 if nch <= 2 else (2 if nch <= 4 else 3)
                    for _ in range(npop):
                        if units:
                            units.pop(0)()
                    # dependency-free weight loads keep the PE activity monitor
                    # from re-throttling the clock during ACT-gated idles
                    for _ in range(FILL_LDW):
                        nc.tensor.ldweights(zero_t[:, :128])
                for item in pending:
                    emit_z(item)
                    for _ in range(2):
                        if units:
                            units.pop(0)()
                while units:
                    units.pop(0)()

    nc.compile()
    return nc


def get_program(S=S_FULL):
    if S not in _prog_cache:
        _prog_cache[S] = _build_program(S)
    return _prog_cache[S]


def make_mask_band():
    """Triangle mask for the last 128 cols of a diagonal chunk:
    col t (relative to the diagonal start) is valid iff t <= r."""
    r = np.arange(128)[:, None]
    t = np.arange(128)[None, :]
    return np.where(t <= r, 0.0, NEG_BIG).astype(np.float32)


def make_core_inputs(x, W_K, W_Q, W_V, W_O, core):
    """Inputs for core = b*4 + g (batch b, head pair a0=2g, a1=2g+1)."""
    b, g = divmod(core, 4)
    a0, a1 = 2 * g, 2 * g + 1
    def tile_pec(m):
        # [E, n] -> [128, EC, n] so the device loads it in one DMA
        ec = m.shape[0] // 128
        return np.ascontiguousarray(
            m.reshape(ec, 128, m.shape[1]).transpose(1, 0, 2)
        ).astype(np.float16)

    xT = tile_pec(x[b].T)
    wk = tile_pec(np.concatenate([W_K[a0].T, W_K[a1].T], axis=1))
    wq = tile_pec(np.concatenate([W_Q[a0].T, W_Q[a1].T], axis=1))
    wv = tile_pec(np.concatenate([W_V[a0].T, W_V[a1].T], axis=1))
    wob = np.ascontiguousarray(np.concatenate([W_O[a0].T, W_O[a1].T], axis=0))
    tri = make_mask_band()
    mskw = np.ones((128, 1152), dtype=np.float16)
    mskw[:, 1024:] = (tri == 0.0).astype(np.float16)
    return {
        "xT": xT, "wk": wk, "wq": wq, "wv": wv,
        "wob": wob.astype(np.float16), "maskw": mskw,
    }


def run_on_cores(inputs, trace=False):
    from concourse.bass_utils import run_bass_kernel_spmd

    nc = get_program()
    in_maps = [
        make_core_inputs(
            inputs["x"], inputs["W_K"], inputs["W_Q"], inputs["W_V"],
            inputs["W_O"], core,
        )
        for core in range(N_CORES)
    ]
    return run_bass_kernel_spmd(
        nc, in_maps, list(range(N_CORES)), trace=trace,
    )


def kernel(x, W_K, W_Q, W_V, W_O):
    x = np.asarray(x, dtype=np.float32)
    W_K = np.asarray(W_K, dtype=np.float32)
    W_Q = np.asarray(W_Q, dtype=np.float32)
    W_V = np.asarray(W_V, dtype=np.float32)
    W_O = np.asarray(W_O, dtype=np.float32)
    res = run_on_cores(
        {"x": x, "W_K": W_K, "W_Q": W_Q, "W_V": W_V, "W_O": W_O}
    )
    out = np.zeros((B, S_FULL, E), dtype=np.float32)
    for b in range(B):
        acc = np.zeros((E, S_FULL), dtype=np.float32)
        for g in range(4):
            acc += res.results[b * 4 + g]["outT"]
        out[b] = acc.T
    return out
